# revision 1
# baseline (speedup 1.0000x reference)
"""Trainium2 Bass kernel for nn_MixtureCDFFlow: invert the per-channel
mixture-of-Gaussians CDF via dyadic bisection + clamped Newton, then
-log(pdf) for the log-det. Batch-sharded over 8 NeuronCores.

Hardcoded problem: B=128, S=2048, D=16, K=8 (fp32).

Per core (32768 tokens), state s = a*x + b lives on partitions (k,d)=128,
tokens along the free dim, processed in two half-passes of 16384 tokens:
  bisection:  F-z = W1^T erf(s) - z  (fp32 matmul + exact hi/lo z split via
              fp32r matmuls), step s +- ac_i from sign(F-z) (ties step up,
              matching the reference).
  newton:     dv = (F-z) * recip_fast(-pdf/a - eps), clamped per iteration
              to a dyadic cap schedule summing to the final bracket
              half-width, so saturated-CDF runaways (z>=0.99999988, forced
              via z:=2 on host) land exactly on the bracket top = ub0 like
              the reference's bisection does.
  outputs:    x = (s - b)/a, nld = -Ln(pdf); Ln(0) = -inf reproduces the
              reference's inf on runaway elements.
"""
import sys

import numpy as np

sys.path.insert(0, "/opt/trn_rl_repo")

import concourse.tile as tile  # noqa: E402
from concourse import bacc, mybir  # noqa: E402
from concourse.bass_utils import run_bass_kernel_spmd  # noqa: E402
from concourse.masks import make_identity  # noqa: E402

f32 = np.float32
AF = mybir.ActivationFunctionType
ALU = mybir.AluOpType
DT = mybir.dt

B, S, D, K = 128, 2048, 16, 8
NCORES = 8
BSH = B // NCORES
NTOK = BSH * S               # 32768 tokens/core
NH = NTOK // 2               # half-pass tokens
TC = 2048                    # chunk free size
NCHUNK = NH // TC
NQ = TC // 512
AH = NH // 128               # tokens per partition in L/OF layout (=128)

N_BIS = 13
N_NEWT = 3
CAP_FRAC = (0.5, 0.25, 0.25)
EPS_PDF = -1e-9
RUN_THRESH = f32(1.0 - 2.0 * 2.0 ** -24)  # 0.99999988

_SQRT2 = f32(np.sqrt(2.0))
_MAGIC = f32(12582912.0)      # 1.5 * 2^23: RNE round-to-int trick

# SC scalar-const columns
C_S0C = 0
C_FCC = 1
C_AC = 2
C_NAC2 = C_AC + N_BIS
C_CAP = C_NAC2 + N_BIS
C_NCAP = C_CAP + N_NEWT
C_BOUT = C_NCAP + N_NEWT
C_INVA = C_BOUT + 1
NCOL = C_INVA + 1

# CONSTF packed fp32 columns: W1 | V2 | B0(rows<16) | V3 | SC
F_W1, F_V2, F_B0, F_V3, F_SC = 0, 128, 256, 384, 400
CF_COLS = 400 + NCOL
# CONSTR packed fp32r columns: Z2(rows<32) | EPS(row 0) | ONES(row 0) | B0
R_Z2, R_EPS, R_ONES = 0, 128, 256
R_B0 = 256 + TC
R_V2 = R_B0 + 128
CR_COLS = R_V2 + 128


def _erf64(v):
    try:
        from scipy.special import erf
        return erf(v.astype(np.float64))
    except Exception:
        import math
        return np.vectorize(math.erf)(v.astype(np.float64))


def _prep(logits, mu, logstd):
    l = np.asarray(logits, f32)
    e = np.exp((l - l.max()).astype(f32)).astype(f32)
    w = (e / e.sum().astype(f32)).astype(f32)
    scale = np.exp(np.asarray(logstd, f32)).astype(f32)
    istd = (f32(1.0) / scale).astype(f32)
    mu = np.asarray(mu, f32)
    a = (istd / _SQRT2).astype(f32)
    b = (-mu * a).astype(f32)
    maxscales = scale.sum(0, dtype=f32)
    lb0 = (mu - f32(10.0) * maxscales).min(0).astype(f32)
    ub0 = (mu + f32(10.0) * maxscales).max(0).astype(f32)
    C = ((lb0 + ub0) * f32(0.5)).astype(f32)
    W = (ub0 - lb0).astype(f32)

    W1 = np.zeros((128, 128), f32)
    V2 = np.zeros((128, 128), f32)
    V3 = np.zeros((128, D), f32)
    for k in range(K):
        rs = slice(k * D, (k + 1) * D)
        V3[rs, :] = np.diag(w[k] * istd[k] / f32(2.0 * np.sqrt(2.0)))
        for kp in range(K):
            cs = slice(kp * D, (kp + 1) * D)
            W1[rs, cs] = np.diag((w[k] * f32(0.5)) * np.ones(D, f32))
            V2[rs, cs] = np.diag(-(w[k] * f32(0.5)) * istd[k] / istd[kp])
    Z2 = np.zeros((128, 128), f32)
    for hl in range(2):
        for kp in range(K):
            Z2[hl * D:(hl + 1) * D, kp * D:(kp + 1) * D] = -np.eye(D, dtype=f32)

    ac = [(a * (W * f32(2.0 ** (-(i + 2)))).astype(f32)).astype(f32).reshape(128)
          for i in range(N_BIS)]
    acl = ac[N_BIS - 1]
    s0 = (a * C + b).astype(f32).reshape(128)

    Fc = np.zeros(D, f32)
    for k in range(K):
        Fc += (w[k] * f32(0.5) * _erf64((a[k] * C + b[k])).astype(f32)).astype(f32)

    B0 = np.zeros((16, 128), f32)
    for kp in range(K):
        B0[:, kp * D:(kp + 1) * D] = np.diag(
            (-f32(2.0)) * ac[0][kp * D:(kp + 1) * D])

    SC = np.zeros((128, NCOL), f32)
    SC[:, C_S0C] = s0 + ac[0]
    SC[:, C_FCC] = np.tile(Fc, 8)
    for i in range(N_BIS):
        SC[:, C_AC + i] = ac[i]
        SC[:, C_NAC2 + i] = f32(-2.0) * ac[i]
    for j in range(N_NEWT):
        SC[:, C_CAP + j] = acl * f32(CAP_FRAC[j])
        SC[:, C_NCAP + j] = -(acl * f32(CAP_FRAC[j]))
    SC[0:D, C_BOUT] = b[0]
    SC[0:D, C_INVA] = (f32(1.0) / a[0]).astype(f32)

    CONSTF = np.zeros((128, CF_COLS), f32)
    CONSTF[:, F_W1:F_W1 + 128] = W1
    CONSTF[:, F_V2:F_V2 + 128] = V2
    CONSTF[0:16, F_B0:F_B0 + 128] = B0
    CONSTF[:, F_V3:F_V3 + D] = V3
    CONSTF[:, F_SC:F_SC + NCOL] = SC

    CONSTR = np.zeros((128, CR_COLS), f32)
    for rep in range(4):
        CONSTR[rep * 32:(rep + 1) * 32, R_Z2:R_Z2 + 128] = Z2[0:32]
    CONSTR[0, R_EPS:R_EPS + 128] = EPS_PDF
    CONSTR[0, R_ONES:R_ONES + TC] = 1.0
    CONSTR[0:16, R_B0:R_B0 + 128] = B0
    CONSTR[:, R_V2:R_V2 + 128] = V2
    return dict(CONSTF=CONSTF, CONSTR=CONSTR)


def _build_nc(nrep=1):
    nc = bacc.Bacc()
    z_in = nc.dram_tensor("z_in", [NTOK, D], DT.float32, kind="ExternalInput")
    cf_d = nc.dram_tensor("CONSTF", [128, CF_COLS], DT.float32, kind="ExternalInput")
    cr_d = nc.dram_tensor("CONSTR", [128, CR_COLS], DT.float32, kind="ExternalInput")
    x_out = nc.dram_tensor("x_out", [NTOK, D], DT.float32, kind="ExternalOutput")
    nld_out = nc.dram_tensor("nld_out", [NTOK, D], DT.float32, kind="ExternalOutput")

    with tile.TileContext(nc) as tc:
        with (
            tc.tile_pool(name="const", bufs=1) as cp,
            tc.tile_pool(name="state", bufs=1) as st,
            tc.tile_pool(name="work", bufs=2) as wk,
            tc.tile_pool(name="once", bufs=1) as w1p,
            tc.tile_pool(name="ps1", bufs=2, space="PSUM") as ps1,
            tc.tile_pool(name="ps2", bufs=2, space="PSUM") as ps2,
            tc.tile_pool(name="pst", bufs=2, space="PSUM") as pst,
        ):
            CF = cp.tile([128, CF_COLS], DT.float32)
            CR = cp.tile([128, CR_COLS], DT.float32r)
            nc.gpsimd.dma_start(CF, cf_d[:, :])
            nc.gpsimd.dma_start(CR, cr_d[:, :])   # cast fp32 -> fp32r
            ident = cp.tile([128, 128], DT.float32)
            make_identity(nc, ident)

            W1 = CF[:, F_W1:F_W1 + 128]
            V2 = CF[:, F_V2:F_V2 + 128]
            B0 = CF[0:16, F_B0:F_B0 + 128]
            V3 = CF[:, F_V3:F_V3 + D]
            SCc = lambda col: CF[:, F_SC + col:F_SC + col + 1]
            SCc16 = lambda col: CF[0:D, F_SC + col:F_SC + col + 1]
            Z2rep = lambda c: CR[(c % 3) * 32:(c % 3) * 32 + 32, R_Z2:R_Z2 + 128]
            V2r = CR[:, R_V2:R_V2 + 128]
            EPS = CR[0:1, R_EPS:R_EPS + 128]
            ONES = CR[0:1, R_ONES:R_ONES + TC]

            for h in [hh for _ in range(nrep) for hh in range(2)]:
                zview = z_in[h * NH:(h + 1) * NH, :].rearrange(
                    "(p a) d -> p (a d)", p=128)
                L = w1p.tile([128, AH * D], DT.float32, tag="L")
                nc.sync.dma_start(L, zview)
                Lv = L[:, :].rearrange("p (a d) -> p a d", d=D)

                # transpose in (tau = a*128 + p), z-0.5, fold bisection
                # step 0, and build the fp32r hi/lo z split -- all in
                # base-partition-0 staging, placed into zs by cast-DMA.
                s = st.tile([128, NH], DT.float32, tag="s")
                zs = [st.tile([96, TC], DT.float32r, tag=f"zs{t}",
                              name=f"zs{t}_{h}") for t in range(3)]
                zhc = w1p.tile([D, TC], DT.float32, tag="OFX")
                zlc = w1p.tile([D, TC], DT.float32, tag="OFN")
                zcc = w1p.tile([D, TC], DT.float32, tag="zcc")
                for blk in range(NH // 512):
                    c, qq = blk // 4, blk % 4
                    ptz = pst.tile([16, 512], DT.float32, tag="ptz")
                    for q in range(4):
                        nc.tensor.transpose(
                            ptz[:, q * 128:(q + 1) * 128],
                            Lv[:, blk * 4 + q, :], ident)
                    zc0 = wk.tile([D, 512], DT.float32, tag="TR")
                    nc.vector.tensor_scalar(zc0, ptz, 0.5, None, ALU.subtract)
                    m0 = wk.tile([D, 512], DT.float32, tag="TA")
                    nc.vector.tensor_scalar(
                        m0, zc0, CF[0:D, F_SC + C_FCC:F_SC + C_FCC + 1],
                        None, ALU.is_lt)
                    p0 = ps1.tile([128, 512], DT.float32, tag="p1")
                    nc.tensor.matmul(p0, B0, m0, start=True, stop=True)
                    nc.vector.tensor_scalar(
                        s[:, blk * 512:(blk + 1) * 512], p0,
                        SCc(C_S0C), None, ALU.add)
                    nc.vector.tensor_copy(
                        zcc[:, qq * 512:(qq + 1) * 512], zc0)
                    if qq == 3:
                        nc.vector.tensor_scalar(
                            zlc, zcc, 4096.0, float(_MAGIC), ALU.mult, ALU.add)
                        nc.vector.tensor_scalar(
                            zhc, zlc, float(_MAGIC), 2.0 ** -12,
                            ALU.subtract, ALU.mult)
                        nc.vector.tensor_tensor(zlc, zcc, zhc, ALU.subtract)
                        r0 = (c % 3) * 32
                        nc.gpsimd.dma_start(zs[c // 3][r0:r0 + D, :], zhc)
                        nc.gpsimd.dma_start(zs[c // 3][r0 + D:r0 + 32, :], zlc)

                # dyadic bisection
                for i in range(1, N_BIS):
                    for c in range(NCHUNK):
                        scs = s[:, c * TC:(c + 1) * TC]
                        zmov = zs[c // 3][(c % 3) * 32:(c % 3) * 32 + 32, :]
                        E = wk.tile([128, TC], DT.float32, tag="TA")
                        nc.scalar.activation(E, scs, AF.Erf)
                        sgn = wk.tile([128, TC], DT.float32, tag="TB")
                        for q in range(NQ):
                            sl = slice(q * 512, (q + 1) * 512)
                            p1 = ps1.tile([128, 512], DT.float32, tag="p1")
                            nc.tensor.matmul(p1, W1, E[:, sl],
                                             start=True, stop=False)
                            nc.tensor.matmul(p1, Z2rep(c), zmov[:, sl],
                                             start=False, stop=True)
                            nc.scalar.activation(sgn[:, sl], p1, AF.Sign)
                        nc.vector.tensor_scalar(
                            sgn, sgn, 0.0, SCc(C_NAC2 + i), ALU.max, ALU.mult)
                        nc.vector.affine_then_add(
                            scs, sgn, scs, 1.0, SCc(C_AC + i))

                # Newton with dyadic caps (chunk pairs batch ACT table sets)
                for j in range(N_NEWT):
                    for c0 in range(0, NCHUNK, 2):
                        Es, Gs = {}, {}
                        for c in (c0, c0 + 1):
                            E = wk.tile([128, TC], DT.float32, tag="TA",
                                        name=f"E_{h}_{j}_{c}")
                            nc.scalar.activation(E, s[:, c * TC:(c + 1) * TC],
                                                 AF.Erf)
                            Es[c] = E
                        for c in (c0, c0 + 1):
                            G = wk.tile([128, TC], DT.float32r, tag="TB",
                                        name=f"Gr_{h}_{j}_{c}")
                            nc.scalar.activation(G, s[:, c * TC:(c + 1) * TC],
                                                 AF.Derivative_Erf)
                            Gs[c] = G
                        for c in (c0, c0 + 1):
                            scs = s[:, c * TC:(c + 1) * TC]
                            zmov = zs[c // 3][(c % 3) * 32:(c % 3) * 32 + 32, :]
                            E, G = Es[c], Gs[c]
                            R = wk.tile([128, TC], DT.float32, tag="TR",
                                        name=f"R_{h}_{j}_{c}")
                            for q in range(NQ):
                                sl = slice(q * 512, (q + 1) * 512)
                                p1 = ps1.tile([128, 512], DT.float32, tag="p1")
                                nc.tensor.matmul(p1, W1, E[:, sl],
                                                 start=True, stop=False)
                                nc.tensor.matmul(p1, Z2rep(c), zmov[:, sl],
                                                 start=False, stop=True)
                                p2 = ps2.tile([128, 512], DT.float32, tag="p2")
                                nc.tensor.matmul(p2, V2r, G[:, sl],
                                                 start=True, stop=False)
                                nc.tensor.matmul(p2, EPS, ONES[:, sl],
                                                 start=False, stop=True)
                                nc.vector.reciprocal_approx_fast(R[:, sl], p2)
                                nc.vector.tensor_tensor(R[:, sl], p1, R[:, sl],
                                                        ALU.mult)
                            nc.vector.tensor_scalar(
                                R, R, SCc(C_CAP + j), SCc(C_NCAP + j),
                                ALU.min, ALU.max)
                            nc.gpsimd.tensor_tensor(scs, scs, R, ALU.add)

                # logdet + outputs (transpose back to token-major)
                OFX = w1p.tile([128, AH * D], DT.float32, tag="OFX")
                OFN = w1p.tile([128, AH * D], DT.float32, tag="OFN")
                for c0 in range(0, NCHUNK, 2):
                    Gs = {}
                    for c in (c0, c0 + 1):
                        G = wk.tile([128, TC], DT.float32, tag="TA",
                                    name=f"Gl_{h}_{c}")
                        nc.scalar.activation(G, s[:, c * TC:(c + 1) * TC],
                                             AF.Derivative_Erf)
                        Gs[c] = G
                    for c in (c0, c0 + 1):
                        scs = s[:, c * TC:(c + 1) * TC]
                        G = Gs[c]
                        nld = wk.tile([D, TC], DT.float32, tag="TB",
                                      name=f"nld_{h}_{c}")
                        for q in range(NQ):
                            sl = slice(q * 512, (q + 1) * 512)
                            p3 = ps1.tile([16, 512], DT.float32, tag="p1")
                            nc.tensor.matmul(p3, V3, G[:, sl], start=True, stop=True)
                            nc.scalar.activation(nld[:, sl], p3, AF.Ln)
                        xo = wk.tile([D, TC], DT.float32, tag="TR",
                                     name=f"xo_{h}_{c}")
                        nc.vector.tensor_scalar(
                            xo, scs[0:D, :], SCc16(C_BOUT), SCc16(C_INVA),
                            ALU.subtract, ALU.mult)
                        for hf in range(2):
                            pox = pst.tile([128, 128], DT.float32, tag="pot")
                            for lt in range(8):
                                tt = hf * 8 + lt
                                nc.tensor.transpose(
                                    pox[:, lt * D:(lt + 1) * D],
                                    xo[:, tt * 128:(tt + 1) * 128],
                                    ident[0:D, 0:D])
                            o0 = c * 256 + hf * 128
                            nc.vector.tensor_copy(OFX[:, o0:o0 + 128], pox)
                            pon = pst.tile([128, 128], DT.float32, tag="pot")
                            for lt in range(8):
                                tt = hf * 8 + lt
                                nc.tensor.transpose(
                                    pon[:, lt * D:(lt + 1) * D],
                                    nld[:, tt * 128:(tt + 1) * 128],
                                    ident[0:D, 0:D])
                            nc.vector.tensor_scalar(
                                OFN[:, o0:o0 + 128], pon, -1.0, None, ALU.mult)
                for od, OF in ((x_out, OFX), (nld_out, OFN)):
                    oview = od[h * NH:(h + 1) * NH, :].rearrange(
                        "(p a) d -> p (a d)", p=128)
                    nc.sync.dma_start(oview, OF)
    nc.finalize()
    return nc


_NC_CACHE = {}


def _get_nc():
    if "nc" not in _NC_CACHE:
        _NC_CACHE["nc"] = _build_nc()
    return _NC_CACHE["nc"]


def kernel(z, logits, mu, logstd):
    z = np.asarray(z, f32)
    consts = _prep(logits, mu, logstd)
    zp = np.where(z >= RUN_THRESH, f32(2.0), z).astype(f32)

    in_maps = []
    for core in range(NCORES):
        zi = np.ascontiguousarray(zp[core * BSH:(core + 1) * BSH].reshape(NTOK, D))
        in_maps.append(dict(z_in=zi, **consts))

    res = run_bass_kernel_spmd(_get_nc(), in_maps, core_ids=list(range(NCORES)))
    x = np.empty((B, S, D), f32)
    nld = np.empty((B, S, D), f32)
    for core in range(NCORES):
        r = res.results[core]
        x[core * BSH:(core + 1) * BSH] = r["x_out"].reshape(BSH, S, D)
        nld[core * BSH:(core + 1) * BSH] = r["nld_out"].reshape(BSH, S, D)
    nld = np.where(z >= RUN_THRESH, np.float32(np.inf), nld).astype(f32)
    return x, nld



# revision 18
# speedup vs baseline: 2.8987x; 2.8987x over previous
"""Trainium2 Bass kernel for nn_MixtureCDFFlow: invert the per-channel
mixture-of-Gaussians CDF, then -log(pdf) for the log-det. Batch-sharded
over 8 NeuronCores.

v2: replaces the 13-round dyadic bisection with a direct analytic seed
  u  = sign(z-1/2) * sqrt(-ln(4 z (1-z)))          (exact in both tails)
  x0 = P_d(u)   (per-channel deg-12 poly, host-fitted to the true inverse)
followed by 2 capped Newton steps on the exact fp32 F(x)-z residual
(fp32 W1*erf matmul + hi/lo-split z subtraction via fp32r matmuls), and
the same -Ln(pdf) logdet as v1. Runaway z>=1-2^-23 handled on host
(x:=ub0, nld:=inf), mirroring the reference's saturated-CDF behaviour.

Hardcoded problem: B=128, S=2048, D=16, K=8 (fp32).

Layouts per core (32768 tokens, two half-passes of NH=16384):
  zpack [128,2048]/half: partition (g,d) = chunk-g row-block, free = token
        within chunk; holds c = z-1/2 (exact). Seed runs full-width here.
  s     [128,2048]/chunk: partition (k,d), free = token; Newton state.
  zphi/zplo [128,2048]/half: hi/lo split of c (2^-12 grid / remainder),
        consumed by partition-windowed fp32r matmuls (Z2rep slices).
"""
import sys

import numpy as np

sys.path.insert(0, "/opt/trn_rl_repo")

import concourse.tile as tile  # noqa: E402
from concourse import bacc, mybir  # noqa: E402
from concourse.bass_utils import run_bass_kernel_spmd  # noqa: E402
from concourse.masks import make_identity  # noqa: E402

f32 = np.float32
AF = mybir.ActivationFunctionType
ALU = mybir.AluOpType
DT = mybir.dt

B, S, D, K = 128, 2048, 16, 8
NCORES = 8
BSH = B // NCORES
NTOK = BSH * S               # 32768 tokens/core
NH = NTOK // 2               # half-pass tokens (16384)
TC = 2048                    # chunk free size
NCHUNK = NH // TC            # 8
NQ = TC // 512               # 4
AH = NH // 128               # 128 cols per partition in L layout

NDEG = 12                    # seed poly degree
CAP1 = 0.5                   # Newton-1 clamp (x units)
CAP2 = 0.15                  # Newton-2 clamp
EPS_PDF = -1e-9
RUN_THRESH = f32(1.0 - 2.0 * 2.0 ** -24)  # 0.99999988

_SQRT2 = f32(np.sqrt(2.0))
_MAGIC = f32(12582912.0)      # 1.5 * 2^23: RNE round-to-int trick

# SC scalar-const columns (CONSTF fp32)
C_A = 0                      # a_kd
C_B = 1                      # b_kd
C_CAP1, C_NCAP1, C_CAP2, C_NCAP2 = 2, 3, 4, 5
C_BOUT, C_INVA = 6, 7
C_H = 8                      # horner coeffs c0..c12 as (g,d)-replicated
NCOL = C_H + NDEG + 1

F_W1, F_SC = 0, 128
CF_COLS = F_SC + NCOL
# CONSTR fp32r columns: Z2big[c]*8 | B1big[c]*8 | V2 | V3 | EPS | ONES
R_Z2 = 0                      # 8 * 128 (also used negated for the bcast)
R_V2 = 8 * 128
R_V3 = R_V2 + 128
R_EPS = R_V3 + D
R_ONES = R_EPS + 128          # EPS lhsT row spans 128 output partitions
CR_COLS = R_ONES + 512


def _erf64(v):
    try:
        from scipy.special import erf
        return erf(np.asarray(v, np.float64))
    except Exception:
        import math
        return np.vectorize(math.erf)(np.asarray(v, np.float64))


def _fit_polys(w, mu, scale):
    """Per-channel deg-NDEG poly fit of x = F_d^{-1}(z) against
    u = sign(z-.5)*sqrt(-ln(4 z(1-z))), via x-grid sampling (no root
    finding) + uniform-u resampling + IRLS toward minimax."""
    coefs = np.zeros((D, NDEG + 1))
    w64 = w.astype(np.float64)
    for d in range(D):
        m64 = mu[:, d].astype(np.float64)
        s64 = scale[:, d].astype(np.float64)
        xlo = (m64 - 6.5 * s64).min()
        xhi = (m64 + 6.5 * s64).max()
        xg = np.linspace(xlo, xhi, 4001)
        F = (w64[None, :] * 0.5 *
             (1.0 + _erf64((xg[:, None] - m64[None, :]) / (s64[None, :] * np.sqrt(2.0))))).sum(1)
        F = np.clip(F, 1e-300, 1.0 - 1e-16)
        q = 4.0 * F * (1.0 - F)
        with np.errstate(divide="ignore"):
            u = np.sqrt(np.maximum(0.0, -np.log(q)))
        u = np.where(F >= 0.5, u, -u)
        # strictly increasing in x; resample to uniform u
        us = np.linspace(-3.84, 3.84, 2501)
        xs = np.interp(us, u, xg)
        wt = np.ones_like(us)
        V = np.polynomial.polynomial.polyvander(us, NDEG)
        c = None
        for _ in range(8):
            c, *_ = np.linalg.lstsq(V * wt[:, None], xs * wt, rcond=None)
            r = np.abs(V @ c - xs)
            wt = wt * (r + 1e-7 * (r.max() + 1e-30)) ** 0.5
            wt /= wt.max()
        coefs[d] = c
    return coefs


def _prep(logits, mu, logstd):
    l = np.asarray(logits, f32)
    e = np.exp((l - l.max()).astype(f32)).astype(f32)
    w = (e / e.sum().astype(f32)).astype(f32)
    scale = np.exp(np.asarray(logstd, f32)).astype(f32)
    istd = (f32(1.0) / scale).astype(f32)
    mu = np.asarray(mu, f32)
    a = (istd / _SQRT2).astype(f32)          # [K,D]
    b = (-mu * a).astype(f32)

    coefs = _fit_polys(w, mu, scale)         # [D, NDEG+1]

    W1 = np.zeros((128, 128), f32)
    V2 = np.zeros((128, 128), f32)
    V3 = np.zeros((128, D), f32)
    for k in range(K):
        rs = slice(k * D, (k + 1) * D)
        V3[rs, :] = np.diag(w[k] * istd[k] / f32(2.0 * np.sqrt(2.0)))
        for kp in range(K):
            cs = slice(kp * D, (kp + 1) * D)
            W1[rs, cs] = np.diag((w[k] * f32(0.5)) * np.ones(D, f32))
            V2[rs, cs] = np.diag(-(w[k] * f32(0.5)) * istd[k] / istd[kp])
    # per-chunk selector blocks: lhsT [128,128] nonzero only at rows of
    # chunk c (g==c), cols (k,d'): +/- delta_dd'. Full-128 rhs keeps the
    # matmul base partition at 0 (HW restriction) at identical cycle cost.
    Z2big = np.zeros((128, 8 * 128), f32)
    for g in range(8):
        for k in range(K):
            Z2big[g * D:(g + 1) * D, g * 128 + k * D:g * 128 + (k + 1) * D] = \
                -np.eye(D, dtype=f32)

    SC = np.zeros((128, NCOL), f32)
    SC[:, C_A] = -a.reshape(128)   # bcast uses Z2big = -B1big
    SC[:, C_B] = b.reshape(128)
    SC[:, C_CAP1] = f32(CAP1) * a.reshape(128)
    SC[:, C_NCAP1] = -f32(CAP1) * a.reshape(128)
    SC[:, C_CAP2] = f32(CAP2) * a.reshape(128)
    SC[:, C_NCAP2] = -f32(CAP2) * a.reshape(128)
    SC[0:D, C_BOUT] = b[0]
    SC[0:D, C_INVA] = (f32(1.0) / a[0]).astype(f32)
    for j in range(NDEG + 1):
        SC[:, C_H + j] = np.tile(coefs[:, j].astype(f32), 8)

    CONSTF = np.zeros((128, CF_COLS), f32)
    CONSTF[:, F_W1:F_W1 + 128] = W1
    CONSTF[:, F_SC:F_SC + NCOL] = SC

    CONSTR = np.zeros((128, CR_COLS), f32)
    CONSTR[:, R_Z2:R_Z2 + 8 * 128] = Z2big
    CONSTR[:, R_V2:R_V2 + 128] = V2
    CONSTR[:, R_V3:R_V3 + D] = V3
    CONSTR[0, R_EPS:R_EPS + 128] = EPS_PDF
    CONSTR[0, R_ONES:R_ONES + 512] = 1.0
    return dict(CONSTF=CONSTF, CONSTR=CONSTR)


def _build_nc(nrep=1):
    nc = bacc.Bacc()
    z_in = nc.dram_tensor("z_in", [NTOK, D], DT.float32, kind="ExternalInput")
    cf_d = nc.dram_tensor("CONSTF", [128, CF_COLS], DT.float32, kind="ExternalInput")
    cr_d = nc.dram_tensor("CONSTR", [128, CR_COLS], DT.float32, kind="ExternalInput")
    x_out = nc.dram_tensor("x_out", [NTOK, D], DT.float32, kind="ExternalOutput")
    nld_out = nc.dram_tensor("nld_out", [NTOK, D], DT.float32, kind="ExternalOutput")

    with tile.TileContext(nc) as tc:
        with (
            tc.tile_pool(name="const", bufs=1) as cp,
            tc.tile_pool(name="zpa", bufs=1) as zpa,     # zpack
            tc.tile_pool(name="zpb", bufs=1) as zpb,     # zphi/zplo
            tc.tile_pool(name="zcp", bufs=1) as zcp,     # zcc staging
            tc.tile_pool(name="sd", bufs=1) as sd,       # us/hi/lo
            tc.tile_pool(name="sp", bufs=1) as sp,       # per-chunk newton state
            tc.tile_pool(name="wk", bufs=2) as wk,       # E/G/dv + seed tmps
            tc.tile_pool(name="io", bufs=1) as io,       # L / OFX / OFN
            tc.tile_pool(name="psb", bufs=2, space="PSUM") as psb,
            tc.tile_pool(name="ps1", bufs=2, space="PSUM") as ps1,
            tc.tile_pool(name="ps2", bufs=2, space="PSUM") as ps2,
        ):
            CF = cp.tile([128, CF_COLS], DT.float32)
            CR = cp.tile([128, CR_COLS], DT.float32r)
            nc.gpsimd.dma_start(CF, cf_d[:, :])
            nc.gpsimd.dma_start(CR, cr_d[:, :])
            ident = cp.tile([128, 128], DT.float32)
            make_identity(nc, ident)

            W1 = CF[:, F_W1:F_W1 + 128]
            SCc = lambda col: CF[:, F_SC + col:F_SC + col + 1]
            SCc16 = lambda col: CF[0:D, F_SC + col:F_SC + col + 1]
            V2r = CR[:, R_V2:R_V2 + 128]
            V3r = CR[:, R_V3:R_V3 + D]
            EPS = CR[0:1, R_EPS:R_EPS + 128]
            ONES = CR[0:1, R_ONES:R_ONES + 512]

            for h in [hh for _ in range(nrep) for hh in range(2)]:
                zview = z_in[h * NH:(h + 1) * NH, :].rearrange(
                    "(p a) d -> p (a d)", p=128)
                L = io.tile([128, AH * D], DT.float32, tag="L")
                nc.sync.dma_start(L, zview)
                Lv = L[:, :].rearrange("p (a d) -> p a d", d=D)

                # ---- input: transpose to (g,d)-packed c = z-0.5 ----
                zpack = zpa.tile([128, TC], DT.float32, tag="zpack",
                                name=f"zpack_{h}")
                for blk in range(NH // 512):
                    c, qq = blk // 4, blk % 4
                    ptz = psb.tile([16, 512], DT.float32, tag="xb")
                    for q in range(4):
                        nc.tensor.transpose(
                            ptz[:, q * 128:(q + 1) * 128],
                            Lv[:, blk * 4 + q, :], ident)
                    if qq == 0:
                        zcc = zcp.tile([D, TC], DT.float32, tag="zcc",
                                       name=f"zcc_{h}_{c}")
                    nc.vector.tensor_scalar(
                        zcc[:, qq * 512:(qq + 1) * 512],
                        ptz, 0.5, None, ALU.subtract)
                    if qq == 3:
                        # engine partition windows must be 32-aligned; DMA
                        # has no such restriction -> place rows via DMA
                        eng = nc.sync if c % 2 == 0 else nc.gpsimd
                        eng.dma_start(zpack[c * D:(c + 1) * D, :], zcc)

                # ---- seed: u then x0 = P_d(u) (Estrin 2-chain) ----
                f2 = wk.tile([128, TC], DT.float32, tag="TA", name=f"f2_{h}")
                nc.vector.tensor_scalar(f2, zpack, -1.0, 0.5, ALU.mult, ALU.add)
                f1 = wk.tile([128, TC], DT.float32, tag="TB", name=f"f1_{h}")
                nc.gpsimd.tensor_scalar(f1, zpack, 0.5, None, ALU.add)
                q4 = wk.tile([128, TC], DT.float32, tag="TR", name=f"q4_{h}")
                nc.vector.tensor_tensor(q4, f1, f2, ALU.mult)
                nc.gpsimd.tensor_scalar(q4, q4, 1e-38, None, ALU.max)
                Lq = wk.tile([128, TC], DT.float32, tag="TA", name=f"Lq_{h}")
                nc.scalar.activation(Lq, q4, AF.Ln, scale=4.0)
                nc.vector.tensor_scalar(Lq, Lq, -1.0, 0.0, ALU.mult, ALU.max)
                u = wk.tile([128, TC], DT.float32, tag="TB", name=f"u_{h}")
                nc.scalar.activation(u, Lq, AF.Sqrt)
                sg = wk.tile([128, TC], DT.float32, tag="TR", name=f"sg_{h}")
                nc.scalar.activation(sg, zpack, AF.Sign)
                us = sd.tile([128, TC], DT.float32, tag="us", name=f"us_{h}")
                nc.vector.tensor_tensor(us, u, sg, ALU.mult)
                us2 = wk.tile([128, TC], DT.float32, tag="TA", name=f"us2_{h}")
                nc.scalar.activation(us2, us, AF.Square)
                us3 = wk.tile([128, TC], DT.float32, tag="TB", name=f"us3_{h}")
                nc.vector.tensor_tensor(us3, us2, us, ALU.mult)
                us6 = wk.tile([128, TC], DT.float32, tag="TR", name=f"us6_{h}")
                nc.scalar.activation(us6, us3, AF.Square)
                # hi chain (DVE): c12..c6 ; lo chain (Pool): c5..c0
                hi = sd.tile([128, TC], DT.float32, tag="hi", name=f"hi_{h}")
                nc.vector.tensor_scalar(hi, us, SCc(C_H + NDEG),
                                        SCc(C_H + NDEG - 1), ALU.mult, ALU.add)
                for j in range(NDEG - 2, 5, -1):
                    nc.vector.tensor_tensor(hi, hi, us, ALU.mult)
                    nc.vector.tensor_scalar(hi, hi, SCc(C_H + j), None, ALU.add)
                lo = sd.tile([128, TC], DT.float32, tag="lo", name=f"lo_{h}")
                nc.gpsimd.tensor_scalar(lo, us, SCc(C_H + 5), SCc(C_H + 4),
                                        ALU.mult, ALU.add)
                for j in range(3, -1, -1):
                    nc.vector.tensor_tensor(lo, lo, us, ALU.mult)
                    nc.gpsimd.tensor_scalar(lo, lo, SCc(C_H + j), None, ALU.add)
                nc.vector.tensor_tensor(hi, hi, us6, ALU.mult)
                nc.vector.tensor_tensor(hi, hi, lo, ALU.add)
                x0 = sd.tile([128, TC], DT.float32r, tag="x0r",
                             name=f"x0r_{h}")
                nc.gpsimd.dma_start(x0, hi)        # cast fp32 -> fp32r

                # ---- hi/lo split of c (full width; emitted late so the
                # 1-buf zphi/zplo reuse stall lands after half-0's N2) ----
                zphi_f = wk.tile([128, TC], DT.float32, tag="TA",
                                 name=f"zphif_{h}")
                zplo_f = wk.tile([128, TC], DT.float32, tag="TB",
                                 name=f"zplof_{h}")
                nc.gpsimd.tensor_scalar(zplo_f, zpack, 4096.0, float(_MAGIC),
                                        ALU.mult, ALU.add)
                nc.vector.tensor_scalar(zphi_f, zplo_f, float(_MAGIC), 2.0 ** -12,
                                        ALU.subtract, ALU.mult)
                nc.vector.tensor_tensor(zplo_f, zpack, zphi_f, ALU.subtract)
                zphi = zpb.tile([128, TC], DT.float32r, tag="zphi",
                                name=f"zphi_{h}")
                zplo = zpb.tile([128, TC], DT.float32r, tag="zplo",
                                name=f"zplo_{h}")
                nc.gpsimd.dma_start(zphi, zphi_f)  # cast fp32 -> fp32r
                nc.gpsimd.dma_start(zplo, zplo_f)

                # ---- broadcast x0 -> s0 = a*x0 + b on (k,d) ----
                ss = []
                for c in range(NCHUNK):
                    B1c = CR[:, R_Z2 + c * 128:R_Z2 + (c + 1) * 128]
                    s_c = sp.tile([128, TC], DT.float32, tag=f"s{c}",
                                  name=f"s_{h}_{c}")
                    ss.append(s_c)
                    for q in range(NQ):
                        sl = slice(q * 512, (q + 1) * 512)
                        xb = psb.tile([128, 512], DT.float32, tag="xb")
                        nc.tensor.matmul(xb, B1c,
                                         x0[:, sl],
                                         start=True, stop=True)
                        nc.vector.tensor_scalar(s_c[:, sl], xb, SCc(C_A),
                                                SCc(C_B), ALU.mult, ALU.add)

                # ---- 2 capped Newton steps (pair-batched ACT sets) ----
                import os
                NIT = int(os.environ.get("KERNEL_NEWTON_ITERS", "2"))
                for it, (ccol, ncol_) in list(enumerate(
                        ((C_CAP1, C_NCAP1), (C_CAP2, C_NCAP2))))[:NIT]:
                    for c0 in range(0, NCHUNK, 2):
                        Es, Gs = {}, {}
                        for c in (c0, c0 + 1):
                            E = wk.tile([128, TC], DT.float32, tag="TA",
                                        name=f"E_{h}_{it}_{c}")
                            nc.scalar.activation(E, ss[c], AF.Erf)
                            Es[c] = E
                        for c in (c0, c0 + 1):
                            G = wk.tile([128, TC], DT.float32r, tag="TB",
                                        name=f"G_{h}_{it}_{c}")
                            nc.scalar.activation(G, ss[c], AF.Derivative_Erf)
                            Gs[c] = G
                        for c in (c0, c0 + 1):
                            Z2c = CR[:, R_Z2 + c * 128:R_Z2 + (c + 1) * 128]
                            dv = wk.tile([128, TC], DT.float32, tag="TR",
                                         name=f"dv_{h}_{it}_{c}")
                            for q in range(NQ):
                                sl = slice(q * 512, (q + 1) * 512)
                                p1 = ps1.tile([128, 512], DT.float32, tag="p1")
                                nc.tensor.matmul(p1, W1, Es[c][:, sl],
                                                 start=True, stop=False)
                                nc.tensor.matmul(p1, Z2c,
                                                 zphi[:, sl],
                                                 start=False, stop=False)
                                nc.tensor.matmul(p1, Z2c,
                                                 zplo[:, sl],
                                                 start=False, stop=True)
                                p2 = ps2.tile([128, 512], DT.float32, tag="p2")
                                nc.tensor.matmul(p2, V2r,
                                                 Gs[c][:, sl],
                                                 start=True, stop=False)
                                nc.tensor.matmul(p2, EPS, ONES,
                                                 start=False, stop=True)
                                if os.environ.get("KERNEL_DEBUG_P1"):
                                    nc.vector.tensor_copy(dv[:, sl], p1)
                                    nc.vector.tensor_scalar(
                                        p2, p2, 1.0, None, ALU.mult)
                                else:
                                    nc.vector.reciprocal_approx_fast(dv[:, sl], p2)
                                    nc.vector.tensor_tensor(dv[:, sl], p1, dv[:, sl],
                                                            ALU.mult)
                            if not os.environ.get("KERNEL_DEBUG_P1"):
                                nc.gpsimd.tensor_scalar(dv, dv, SCc(ccol),
                                                        SCc(ncol_), ALU.min, ALU.max)
                            nc.gpsimd.tensor_tensor(ss[c], ss[c], dv, ALU.add)

                # ---- logdet + outputs ----
                OFX = io.tile([128, AH * D], DT.float32, tag="L", name=f"OFX_{h}")
                OFN = io.tile([128, AH * D], DT.float32, tag="OFN")
                G3s = {}
                for c in range(NCHUNK):
                    G3 = wk.tile([128, TC], DT.float32r, tag="TA",
                                 name=f"G3_{h}_{c}")
                    nc.scalar.activation(G3, ss[c], AF.Derivative_Erf)
                    G3s[c] = G3
                for c in range(NCHUNK):
                    nld = wk.tile([D, TC], DT.float32, tag="TB",
                                  name=f"nld_{h}_{c}")
                    for q in range(NQ):
                        sl = slice(q * 512, (q + 1) * 512)
                        p3 = ps1.tile([16, 512], DT.float32, tag="p1")
                        nc.tensor.matmul(p3, V3r,
                                         G3s[c][:, sl],
                                         start=True, stop=True)
                        nc.scalar.activation(nld[:, sl], p3, AF.Ln)
                    xo = wk.tile([D, TC], DT.float32, tag="TR",
                                 name=f"xo_{h}_{c}")
                    nc.vector.tensor_scalar(
                        xo, ss[c][0:D, :], SCc16(C_BOUT), SCc16(C_INVA),
                        ALU.subtract, ALU.mult)
                    for hf in range(2):
                        pox = psb.tile([128, 128], DT.float32, tag="xb")
                        for lt in range(8):
                            tt = hf * 8 + lt
                            nc.tensor.transpose(
                                pox[:, lt * D:(lt + 1) * D],
                                xo[:, tt * 128:(tt + 1) * 128],
                                ident[0:D, 0:D])
                        o0 = c * 256 + hf * 128
                        nc.vector.tensor_copy(OFX[:, o0:o0 + 128], pox)
                        pon = psb.tile([128, 128], DT.float32, tag="xb")
                        for lt in range(8):
                            tt = hf * 8 + lt
                            nc.tensor.transpose(
                                pon[:, lt * D:(lt + 1) * D],
                                nld[:, tt * 128:(tt + 1) * 128],
                                ident[0:D, 0:D])
                        nc.vector.tensor_scalar(
                            OFN[:, o0:o0 + 128], pon, -1.0, None, ALU.mult)
                for od, OF in ((x_out, OFX), (nld_out, OFN)):
                    oview = od[h * NH:(h + 1) * NH, :].rearrange(
                        "(p a) d -> p (a d)", p=128)
                    nc.sync.dma_start(oview, OF)
    nc.finalize()
    return nc


_NC_CACHE = {}


def _get_nc():
    if "nc" not in _NC_CACHE:
        _NC_CACHE["nc"] = _build_nc()
    return _NC_CACHE["nc"]


def kernel(z, logits, mu, logstd):
    z = np.asarray(z, f32)
    consts = _prep(logits, mu, logstd)
    zp_ = np.where(z >= RUN_THRESH, f32(2.0), z).astype(f32)

    in_maps = []
    for core in range(NCORES):
        zi = np.ascontiguousarray(zp_[core * BSH:(core + 1) * BSH].reshape(NTOK, D))
        in_maps.append(dict(z_in=zi, **consts))

    res = run_bass_kernel_spmd(_get_nc(), in_maps, core_ids=list(range(NCORES)))
    x = np.empty((B, S, D), f32)
    nld = np.empty((B, S, D), f32)
    for core in range(NCORES):
        r = res.results[core]
        x[core * BSH:(core + 1) * BSH] = r["x_out"].reshape(BSH, S, D)
        nld[core * BSH:(core + 1) * BSH] = r["nld_out"].reshape(BSH, S, D)
    # runaway saturated-CDF elements: reference lands on ub0 with +inf logdet
    scale = np.exp(np.asarray(logstd, f32)).astype(f32)
    maxscales = scale.sum(0, dtype=f32)
    ub0 = (np.asarray(mu, f32) + f32(10.0) * maxscales).max(0).astype(f32)
    run = z >= RUN_THRESH
    x = np.where(run, ub0[None, None, :], x).astype(f32)
    nld = np.where(run, np.float32(np.inf), nld).astype(f32)
    return x, nld
